# revision 1
# baseline (speedup 1.0000x reference)
"""Trainium2 Bass kernel for dense-matmul MoE routing (nn_JaxMoE_26431228740246).

Strategy: expert parallel across 8 NeuronCores (1 expert per core, tokens
replicated).  Each core computes its expert's full SwiGLU MLP over all tokens
plus the router weights for its expert, returning a weighted partial output
outT[e] = (w_e[t] * (silu(x@Wg_e) * (x@Wu_e)) @ Wd_e).T  in [D, T] layout.
Host gathers: out = (sum_e outT[e]).T.

Shapes (hardcoded): T=2048, D=1024, F=4096, E=8, K=2 (top-k renormalized).

All big matmuls run as float32r (full-rate fp32 PE mode).  Activations live in
[f, t] layout so all weight operands stream from DRAM in their natural
row-major layout; x is transposed on-chip once per token-slice via the PE.
"""

import os
import sys

import numpy as np


def _ensure_path():
    for p in (
        "/root/.axon_site",
        "/root/.axon_site/_ro/trn_rl_repo",
        "/root/.axon_site/_ro/pypackages",
        "/opt/trn_rl_repo",
    ):
        if os.path.isdir(p) and p not in sys.path:
            sys.path.append(p)


_ensure_path()

T, D, F, E = 2048, 1024, 4096, 8
TS = 1024           # tokens per slice
NS = T // TS        # 2 slices
NH = TS // 512      # 512-wide halves per slice
FTILES = F // 128   # 32
FH = FTILES // 2    # f-tiles per h phase (16)
DT = D // 128       # 8 d-tiles
FC = 128            # f columns per gate/up weight DMA chunk

_CACHE = {}


def _build():
    import concourse.tile as tile
    from concourse import bacc, mybir
    from concourse.masks import make_identity

    fp32 = mybir.dt.float32
    f32r = mybir.dt.float32r
    Act = mybir.ActivationFunctionType
    Alu = mybir.AluOpType
    from concourse import bass_isa

    nc = bacc.Bacc("TRN2", target_bir_lowering=False, debug=False, num_devices=E)

    x = nc.dram_tensor("x", [T, D], fp32, kind="ExternalInput").ap()
    wr = nc.dram_tensor("wr", [D, E], f32r, kind="ExternalInput").ap()
    wg = nc.dram_tensor("wg", [D, F], f32r, kind="ExternalInput").ap()
    wu = nc.dram_tensor("wu", [D, F], f32r, kind="ExternalInput").ap()
    wd = nc.dram_tensor("wd", [F, D], f32r, kind="ExternalInput").ap()
    sel = nc.dram_tensor("sel", [E, 1], fp32, kind="ExternalInput").ap()
    outT = nc.dram_tensor("outT", [D, T], fp32, kind="ExternalOutput").ap()

    # natural-layout DRAM views with 128-partition inner dims
    x_r = x.rearrange("(to ti) d -> ti to d", ti=128)        # [128, T/128, D]
    wg_r = wg.rearrange("(do di) f -> di do f", di=128)      # [128, 8, F]
    wu_r = wu.rearrange("(do di) f -> di do f", di=128)
    wd_r = wd.rearrange("(fo fi) d -> fi fo d", fi=128)      # [128, 32, D]
    wr_r = wr.rearrange("(do di) e -> di do e", di=128)      # [128, 8, E]

    from contextlib import ExitStack

    with tile.TileContext(nc) as tc, ExitStack() as ctx:
        pconst = ctx.enter_context(tc.tile_pool(name="const", bufs=1))
        pxin = ctx.enter_context(tc.tile_pool(name="xin", bufs=2))
        pxT = ctx.enter_context(tc.tile_pool(name="xT", bufs=1))
        ph = ctx.enter_context(tc.tile_pool(name="h", bufs=1))
        pwgu = ctx.enter_context(tc.tile_pool(name="wgu", bufs=2))
        pwd = ctx.enter_context(tc.tile_pool(name="wd", bufs=2))
        pacc = ctx.enter_context(tc.tile_pool(name="acc", bufs=1))
        posb = ctx.enter_context(tc.tile_pool(name="osb", bufs=2))
        pwb = ctx.enter_context(tc.tile_pool(name="wb", bufs=1))
        prt = ctx.enter_context(tc.tile_pool(name="rt", bufs=4))
        ptmp = ctx.enter_context(tc.tile_pool(name="tmp", bufs=2))
        pmm = ctx.enter_context(tc.tile_pool(name="mm", bufs=6, space="PSUM"))
        ptp = ctx.enter_context(tc.tile_pool(name="tp", bufs=2, space="PSUM"))

        ident = pconst.tile([128, 128], fp32, tag="ident")
        make_identity(nc, ident[:])
        wr_sb = pconst.tile([128, DT, E], f32r, tag="wr")
        nc.sync.dma_start(wr_sb[:], wr_r[:])
        sel_sb = pconst.tile([E, 1], fp32, tag="sel")
        nc.sync.dma_start(sel_sb[:], sel[:])

        for ts in range(NS):
            # ---- transpose x slice -> xT [128(di), 8(do), TS] ----
            xT = pxT.tile([128, DT, TS], f32r, tag="xT")
            for tt in range(TS // 128):
                xa = pxin.tile([128, D], fp32, tag="xa")
                nc.sync.dma_start(xa[:], x_r[:, ts * (TS // 128) + tt, :])
                for do in range(DT):
                    pt = ptp.tile([128, 128], fp32, tag="tp")
                    nc.tensor.transpose(pt[:], xa[:, do * 128 : (do + 1) * 128], ident[:])
                    nc.vector.tensor_copy(xT[:, do, tt * 128 : (tt + 1) * 128], pt[:])

            # ---- router: logitsT[e, t] then top-2 renormalized weights ----
            lg = prt.tile([E, TS], fp32, tag="rt")
            for hf in range(NH):
                prm = pmm.tile([E, 512], fp32, tag="mm")
                for do in range(DT):
                    nc.tensor.matmul(
                        prm[:],
                        wr_sb[:, do, :],
                        xT[:, do, hf * 512 : (hf + 1) * 512],
                        start=(do == 0),
                        stop=(do == DT - 1),
                    )
                nc.vector.tensor_copy(lg[:, hf * 512 : (hf + 1) * 512], prm[:])
            m1 = prt.tile([E, TS], fp32, tag="rt")
            nc.gpsimd.partition_all_reduce(m1[:], lg[:], channels=E, reduce_op=bass_isa.ReduceOp.max)
            eq = prt.tile([E, TS], fp32, tag="rt")
            nc.vector.tensor_tensor(eq[:], lg[:], m1[:], op=Alu.is_equal)
            # eq <- logits with the argmax masked out (in-place)
            nc.vector.scalar_tensor_tensor(eq[:], eq[:], -1e30, lg[:], op0=Alu.mult, op1=Alu.add)
            m2 = prt.tile([E, TS], fp32, tag="rt")
            nc.gpsimd.partition_all_reduce(m2[:], eq[:], channels=E, reduce_op=bass_isa.ReduceOp.max)
            ge = prt.tile([E, TS], fp32, tag="rt")
            nc.vector.tensor_tensor(ge[:], lg[:], m2[:], op=Alu.is_ge)
            # lg <- exp(lg - m1) * ge  (renormalized top-2 numerators, in-place)
            nc.vector.tensor_sub(lg[:], lg[:], m1[:])
            nc.scalar.activation(lg[:], lg[:], Act.Exp)
            nc.vector.tensor_mul(lg[:], lg[:], ge[:])
            dn = prt.tile([E, TS], fp32, tag="rt")
            nc.gpsimd.partition_all_reduce(dn[:], lg[:], channels=E, reduce_op=bass_isa.ReduceOp.add)
            rc = prt.tile([E, TS], fp32, tag="rt")
            nc.vector.reciprocal(rc[:], dn[:])
            nc.vector.tensor_mul(lg[:], lg[:], rc[:])
            nc.vector.tensor_scalar_mul(lg[:], lg[:], sel_sb[:, 0:1])
            wr8 = prt.tile([E, TS], fp32, tag="rt")
            nc.gpsimd.partition_all_reduce(wr8[:], lg[:], channels=E, reduce_op=bass_isa.ReduceOp.add)
            wb = pwb.tile([128, TS], fp32, tag="wb")
            nc.gpsimd.partition_broadcast(wb[:], wr8[0:1, :], channels=128)

            acc = pacc.tile([128, DT, TS], fp32, tag="acc")

            for fh in range(2):
                # ---- A: gate/up -> h for f-tiles [fh*FH, (fh+1)*FH) ----
                h = ph.tile([128, FH, TS], f32r, tag="h")
                for fc in range(FH * 128 // FC):
                    f0 = fh * FH * 128 + fc * FC
                    wg_t = pwgu.tile([128, DT, FC], f32r, tag="wg")
                    nc.sync.dma_start(wg_t[:], wg_r[:, :, f0 : f0 + FC])
                    wu_t = pwgu.tile([128, DT, FC], f32r, tag="wu")
                    nc.sync.dma_start(wu_t[:], wu_r[:, :, f0 : f0 + FC])
                    for fi in range(FC // 128):
                        k = fc * (FC // 128) + fi  # h tile index within fh
                        for hf in range(NH):
                            pg = pmm.tile([128, 512], fp32, tag="mm")
                            for do in range(DT):
                                nc.tensor.matmul(
                                    pg[:],
                                    wg_t[:, do, fi * 128 : (fi + 1) * 128],
                                    xT[:, do, hf * 512 : (hf + 1) * 512],
                                    start=(do == 0),
                                    stop=(do == DT - 1),
                                )
                            pu = pmm.tile([128, 512], fp32, tag="mm")
                            for do in range(DT):
                                nc.tensor.matmul(
                                    pu[:],
                                    wu_t[:, do, fi * 128 : (fi + 1) * 128],
                                    xT[:, do, hf * 512 : (hf + 1) * 512],
                                    start=(do == 0),
                                    stop=(do == DT - 1),
                                )
                            tmp = ptmp.tile([128, 512], fp32, tag="stmp")
                            nc.scalar.activation(tmp[:], pg[:], Act.Silu)
                            nc.vector.tensor_mul(
                                h[:, k, hf * 512 : (hf + 1) * 512], tmp[:], pu[:]
                            )

                # ---- B: partial down-projection over this fh ----
                for dd in range(DT):
                    wd_t = pwd.tile([128, FH, 128], f32r, tag="wd")
                    nc.sync.dma_start(
                        wd_t[:], wd_r[:, fh * FH : (fh + 1) * FH, dd * 128 : (dd + 1) * 128]
                    )
                    for hf in range(NH):
                        po = pmm.tile([128, 512], fp32, tag="mm")
                        for k in range(FH):
                            nc.tensor.matmul(
                                po[:],
                                wd_t[:, k, :],
                                h[:, k, hf * 512 : (hf + 1) * 512],
                                start=(k == 0),
                                stop=(k == FH - 1),
                            )
                        dst = acc[:, dd, hf * 512 : (hf + 1) * 512]
                        if fh == 0:
                            nc.vector.tensor_copy(dst, po[:])
                        else:
                            nc.vector.tensor_add(dst, dst, po[:])

            # ---- scale by router weight, store ----
            for dd in range(DT):
                osb = posb.tile([128, TS], fp32, tag="osb")
                nc.vector.tensor_mul(osb[:], acc[:, dd, :], wb[:])
                nc.sync.dma_start(
                    outT[dd * 128 : (dd + 1) * 128, ts * TS : (ts + 1) * TS], osb[:]
                )

    nc.compile()
    return nc


def _get_nc():
    if "nc" not in _CACHE:
        _CACHE["nc"] = _build()
    return _CACHE["nc"]


def kernel(
    x_TD, w_router_DE, kernel_gating_EDF, kernel_up_proj_EDF, kernel_down_proj_EFD
):
    from concourse.bass_utils import run_bass_kernel_spmd

    x = np.ascontiguousarray(np.asarray(x_TD, dtype=np.float32))
    wr = np.ascontiguousarray(np.asarray(w_router_DE, dtype=np.float32))
    g = np.asarray(kernel_gating_EDF, dtype=np.float32)
    u = np.asarray(kernel_up_proj_EDF, dtype=np.float32)
    d = np.asarray(kernel_down_proj_EFD, dtype=np.float32)

    nc = _get_nc()
    in_maps = []
    for e in range(E):
        sel = np.zeros((E, 1), dtype=np.float32)
        sel[e, 0] = 1.0
        in_maps.append(
            {
                "x": x,
                "wr": wr,
                "wg": np.ascontiguousarray(g[e]),
                "wu": np.ascontiguousarray(u[e]),
                "wd": np.ascontiguousarray(d[e]),
                "sel": sel,
            }
        )

    trace = bool(os.environ.get("BASS_PROF"))
    try:
        res = run_bass_kernel_spmd(nc, in_maps, list(range(E)), trace=trace)
    except Exception:
        if not trace:
            raise
        res = run_bass_kernel_spmd(nc, in_maps, list(range(E)), trace=False)
    _CACHE["last_result"] = res

    out = res.results[0]["outT"].astype(np.float64)
    for e in range(1, E):
        out += res.results[e]["outT"]
    return np.ascontiguousarray(out.T.astype(np.float32))

